# revision 2
# baseline (speedup 1.0000x reference)
"""CrossAttention kernel for 8 Trainium2 NeuronCores.

Reference computation (per batch element b):
    q = ts[b] @ q_w.T + q_b          # [512, 1024]
    k = llm[b] @ k_w.T + k_b         # [2048, 1024]
    v = llm[b] @ v_w.T + v_b         # [2048, 1024]
    per head h (16 heads x 64 dims):
        scores = q_h @ k_h.T / 8     # [512, 2048]
        attn = softmax(scores, -1)
        ctx_h = attn @ v_h           # [512, 64]
    out = ctx @ o_w.T + o_b          # [512, 1024]

Sharding: data-parallel over batch (B=8 -> one element per core), no
collectives.  Per-core kernel keeps everything in "feature-major"
layouts so no on-device transposes are needed:

  QT[j, p]  = q_w @ ts.T  + q_b     (j on partitions, bias per-partition)
  KT[j, s]  = k_w @ llm.T + k_b
  V'[s, j'] = llm @ v_w.T + v_b     (natural layout; bias via broadcast add;
                                     j' = 16 heads x 65 cols, col 64 of each
                                     head block is ones -> softmax denom)
  scoresT_h[s, p] = KT_h.T @ QT_h   (K=64 contraction)
  expT = exp(scoresT / 8)           (no max subtraction: |scores/8| < ~3)
  ctx'_h[0:64, p] = V'_h.T @ expT   (accumulated over s; row 64 = denom)
  ctxT_h = ctx'_h[0:64] * (1/denom) (reciprocal + rank-1 fp32 bcast matmul)
  out[p, j] = ctxT.T @ o_wT + o_b

All matmuls bf16 inputs / fp32 PSUM accumulate.  Host does layout-only
prep (transpose, bf16 cast, bias broadcast).
"""
import numpy as np
import ml_dtypes

D = 1024          # d_model
P = 512           # ts sequence length
S = 2048          # llm sequence length
H = 16            # heads
DH = 64           # head dim
NCORES = 8
NDT = D // 128    # 8 d-tiles
NST = S // 128    # 16 s-tiles
NPT = P // 128    # 4 p-tiles

_BF16 = ml_dtypes.bfloat16

_cached_nc = None


def _build_nc():
    import concourse.tile as tile
    from concourse import bacc, mybir

    f32 = mybir.dt.float32
    bf16 = mybir.dt.bfloat16
    Exp = mybir.ActivationFunctionType.Exp

    nc = bacc.Bacc("TRN2", target_bir_lowering=False, debug=False,
                   num_devices=NCORES)

    tsT = nc.declare_dram_parameter("tsT", [D, P], bf16, isOutput=False)
    llmT = nc.declare_dram_parameter("llmT", [D, S], bf16, isOutput=False)
    qwT = nc.declare_dram_parameter("qwT", [D, D], bf16, isOutput=False)
    kwT = nc.declare_dram_parameter("kwT", [D, D], bf16, isOutput=False)
    vwT = nc.declare_dram_parameter("vwT", [D, D], bf16, isOutput=False)
    owT = nc.declare_dram_parameter("owT", [D, D], bf16, isOutput=False)
    qkb = nc.declare_dram_parameter("qkb", [128, 2 * NDT], f32, isOutput=False)
    vbb = nc.declare_dram_parameter("vbb", [128, D], f32, isOutput=False)
    obb = nc.declare_dram_parameter("obb", [128, D], f32, isOutput=False)
    out = nc.declare_dram_parameter("out", [P, D], f32, isOutput=True)

    with tile.TileContext(nc) as tc:
        _emit(tc, nc, tile, mybir, f32, bf16, Exp,
              tsT, llmT, qwT, kwT, vwT, owT, qkb, vbb, obb, out)
    nc.compile()
    return nc


def _emit(tc, nc, tile, mybir, f32, bf16, Exp,
          tsT, llmT, qwT, kwT, vwT, owT, qkb, vbb, obb, out):
    from contextlib import ExitStack

    with ExitStack() as ctx:
        persist = ctx.enter_context(tc.tile_pool(name="persist", bufs=1))
        wpool = ctx.enter_context(tc.tile_pool(name="wpool", bufs=16))
        expool = ctx.enter_context(tc.tile_pool(name="expool", bufs=6))
        rpool = ctx.enter_context(tc.tile_pool(name="rpool", bufs=2))
        opool = ctx.enter_context(tc.tile_pool(name="opool", bufs=3))

        # ---- constants / biases ----
        qkb_sb = persist.tile([128, 2 * NDT], f32, name="qkb_sb", tag="qkb_sb")
        nc.sync.dma_start(out=qkb_sb, in_=qkb.ap())
        vbb_sb = persist.tile([128, D], f32, name="vbb_sb", tag="vbb_sb")
        nc.sync.dma_start(out=vbb_sb, in_=vbb.ap())
        obb_sb = persist.tile([128, D], f32, name="obb_sb", tag="obb_sb")
        nc.sync.dma_start(out=obb_sb, in_=obb.ap())
        ones64 = persist.tile([1, DH], f32, name="ones64", tag="ones64")
        nc.vector.memset(ones64, 1.0)

        # ---- activations ----
        ts_sb = []
        for d in range(NDT):
            t = persist.tile([128, P], bf16, name=f"ts_sb{d}", tag=f"ts_sb{d}")
            nc.sync.dma_start(out=t, in_=tsT.ap()[d * 128:(d + 1) * 128, :])
            ts_sb.append(t)
        llm_sb = []
        for d in range(NDT):
            t = persist.tile([128, S], bf16, name=f"llm_sb{d}", tag=f"llm_sb{d}")
            nc.sync.dma_start(out=t, in_=llmT.ap()[d * 128:(d + 1) * 128, :])
            llm_sb.append(t)

        # ---- weights (streamed through a shared 16-slot pool) ----
        def load_w(dram, prefix):
            tiles = []
            for d in range(NDT):
                t = wpool.tile([128, D], bf16, name=f"{prefix}{d}", tag="w")
                nc.sync.dma_start(out=t, in_=dram.ap()[d * 128:(d + 1) * 128, :])
                tiles.append(t)
            return tiles

        qw_sb = load_w(qwT, "qw_sb")
        kw_sb = load_w(kwT, "kw_sb")

        # ================= phase A: projections =================
        with tc.tile_pool(name="psA", bufs=6, space="PSUM") as psA:
            # QT[j, p] = q_w @ ts.T + q_b
            qt_sb = []
            for jt in range(NDT):
                ps = psA.tile([128, P], f32, name=f"ps_q{jt}", tag="psA")
                for d in range(NDT):
                    nc.tensor.matmul(
                        ps, lhsT=qw_sb[d][:, jt * 128:(jt + 1) * 128],
                        rhs=ts_sb[d], start=(d == 0), stop=(d == NDT - 1))
                qt = persist.tile([128, P], bf16, name=f"qt_sb{jt}",
                                  tag=f"qt_sb{jt}")
                nc.vector.tensor_scalar_add(qt, ps, qkb_sb[:, jt:jt + 1])
                qt_sb.append(qt)

            vw_sb = load_w(vwT, "vw_sb")

            # KT[j, s] = k_w @ llm.T + k_b
            kt_sb = []
            for jt in range(NDT):
                kt = persist.tile([128, S], bf16, name=f"kt_sb{jt}",
                                  tag=f"kt_sb{jt}")
                for sc in range(S // 512):
                    ps = psA.tile([128, 512], f32, name=f"ps_k{jt}_{sc}",
                                  tag="psA")
                    for d in range(NDT):
                        nc.tensor.matmul(
                            ps, lhsT=kw_sb[d][:, jt * 128:(jt + 1) * 128],
                            rhs=llm_sb[d][:, sc * 512:(sc + 1) * 512],
                            start=(d == 0), stop=(d == NDT - 1))
                    nc.vector.tensor_scalar_add(
                        kt[:, sc * 512:(sc + 1) * 512], ps,
                        qkb_sb[:, NDT + jt:NDT + jt + 1])
                kt_sb.append(kt)

            ow_sb = load_w(owT, "ow_sb")

            # V'[s, h*65 + x]: x<64 -> v_h columns, x=64 -> ones
            vp_sb = []
            for st in range(NST):
                vp = persist.tile([128, H * (DH + 1)], bf16,
                                  name=f"vp_sb{st}", tag=f"vp_sb{st}")
                vp3 = vp.rearrange("p (h x) -> p h x", x=DH + 1)
                nc.vector.memset(vp3[:, :, DH:DH + 1], 1.0)
                for jc in range(2):
                    ps = psA.tile([128, 512], f32, name=f"ps_v{st}_{jc}",
                                  tag="psA")
                    for d in range(NDT):
                        nc.tensor.matmul(
                            ps, lhsT=llm_sb[d][:, st * 128:(st + 1) * 128],
                            rhs=vw_sb[d][:, jc * 512:(jc + 1) * 512],
                            start=(d == 0), stop=(d == NDT - 1))
                    nc.vector.tensor_add(
                        vp3[:, jc * 8:(jc + 1) * 8, 0:DH],
                        ps.rearrange("p (h x) -> p h x", x=DH),
                        vbb_sb[:, jc * 512:(jc + 1) * 512]
                        .rearrange("p (h x) -> p h x", x=DH))
                vp_sb.append(vp)

        # ================= phase B: attention =================
        cx_sb = [None] * NDT
        with tc.tile_pool(name="psS", bufs=2, space="PSUM") as psS, \
             tc.tile_pool(name="psC", bufs=2, space="PSUM") as psC, \
             tc.tile_pool(name="psB", bufs=2, space="PSUM") as psB:
            for h in range(H):
                jt, rs = h // 2, (h % 2) * DH
                # scoresT_h = KT_h.T @ QT_h, two s-tiles per psum tile,
                # exp'd straight out of PSUM by ScalarE.
                exps = []
                for sp in range(NST // 2):
                    ps = psS.tile([128, 1024], f32, name=f"ps_s{h}_{sp}",
                                  tag="psS")
                    for k in range(2):
                        st = sp * 2 + k
                        nc.tensor.matmul(
                            ps[:, k * 512:(k + 1) * 512],
                            lhsT=kt_sb[jt][rs:rs + DH,
                                           st * 128:(st + 1) * 128],
                            rhs=qt_sb[jt][rs:rs + DH, :],
                            start=True, stop=True)
                    et = expool.tile([128, 1024], bf16, name=f"et{h}_{sp}",
                                     tag="et")
                    nc.scalar.activation(et, ps, Exp, bias=0.0, scale=0.125)
                    exps.append(et)
                # ctx'_h: rows 0:64 = unnormalized ctx.T, row 64 = denom
                psc = psC.tile([DH + 1, 512], f32, name=f"ps_c{h}", tag="psC")
                for st in range(NST):
                    nc.tensor.matmul(
                        psc,
                        lhsT=vp_sb[st][:, h * (DH + 1):(h + 1) * (DH + 1)],
                        rhs=exps[st // 2][:, (st % 2) * 512:(st % 2 + 1) * 512],
                        start=(st == 0), stop=(st == NST - 1))
                rc = rpool.tile([1, 512], f32, name=f"rc{h}", tag="rc")
                nc.vector.reciprocal(rc, psc[DH:DH + 1, :])
                psb = psB.tile([DH, 512], f32, name=f"ps_b{h}", tag="psB")
                nc.tensor.matmul(psb, lhsT=ones64, rhs=rc,
                                 start=True, stop=True)
                # DVE may read only one operand from PSUM: stage the
                # broadcast reciprocal in SBUF before the multiply.
                rb = rpool.tile([DH, 512], f32, name=f"rb{h}", tag="rb")
                nc.vector.tensor_copy(rb, psb)
                if rs == 0:
                    cx_sb[jt] = persist.tile([128, P], bf16,
                                             name=f"cx_sb{jt}",
                                             tag=f"cx_sb{jt}")
                nc.vector.tensor_mul(cx_sb[jt][rs:rs + DH, :],
                                     psc[0:DH, :], rb)

        # ================= phase C: output projection =================
        with tc.tile_pool(name="psO", bufs=4, space="PSUM") as psO:
            for pt in range(NPT):
                for jc in range(2):
                    ps = psO.tile([128, 512], f32, name=f"ps_o{pt}_{jc}",
                                  tag="psO")
                    for d in range(NDT):
                        nc.tensor.matmul(
                            ps, lhsT=cx_sb[d][:, pt * 128:(pt + 1) * 128],
                            rhs=ow_sb[d][:, jc * 512:(jc + 1) * 512],
                            start=(d == 0), stop=(d == NDT - 1))
                    ot = opool.tile([128, 512], f32, name=f"ot{pt}_{jc}",
                                    tag="ot")
                    nc.vector.tensor_add(ot, ps, obb_sb[:, jc * 512:(jc + 1) * 512])
                    nc.sync.dma_start(
                        out=out.ap()[pt * 128:(pt + 1) * 128,
                                     jc * 512:(jc + 1) * 512],
                        in_=ot)


def get_nc():
    global _cached_nc
    if _cached_nc is None:
        _cached_nc = _build_nc()
    return _cached_nc


def make_in_maps(ts_features, llm_features, q_w, q_b, k_w, k_b, v_w, v_b,
                 o_w, o_b):
    ts = np.asarray(ts_features, np.float32)
    llm = np.asarray(llm_features, np.float32)
    shared = {
        "qwT": np.ascontiguousarray(np.asarray(q_w, np.float32).T).astype(_BF16),
        "kwT": np.ascontiguousarray(np.asarray(k_w, np.float32).T).astype(_BF16),
        "vwT": np.ascontiguousarray(np.asarray(v_w, np.float32).T).astype(_BF16),
        "owT": np.ascontiguousarray(np.asarray(o_w, np.float32).T).astype(_BF16),
        "qkb": np.ascontiguousarray(np.concatenate(
            [np.asarray(q_b, np.float32).reshape(NDT, 128).T,
             np.asarray(k_b, np.float32).reshape(NDT, 128).T], axis=1)),
        "vbb": np.ascontiguousarray(
            np.broadcast_to(np.asarray(v_b, np.float32), (128, D))),
        "obb": np.ascontiguousarray(
            np.broadcast_to(np.asarray(o_b, np.float32), (128, D))),
    }
    in_maps = []
    for b in range(NCORES):
        m = dict(shared)
        m["tsT"] = np.ascontiguousarray(ts[b].T).astype(_BF16)
        m["llmT"] = np.ascontiguousarray(llm[b].T).astype(_BF16)
        in_maps.append(m)
    return in_maps


def kernel(**inputs):
    from concourse.bass_utils import run_bass_kernel_spmd

    nc = get_nc()
    in_maps = make_in_maps(**inputs)
    res = run_bass_kernel_spmd(nc, in_maps, list(range(NCORES)))
    return np.stack([res.results[i]["out"] for i in range(NCORES)], axis=0)
